# revision 17
# baseline (speedup 1.0000x reference)
"""CORAL loss kernel for Trainium2 (8 NeuronCores, Bass/Tile).

Strategy (data-parallel over bz, per sharding hint):
  - Shard features [32, 4096, 256] along bz: 4 batch elements per core.
  - Host casts features to fp8 e4m3 (TRN float8e4, max +-240; values are
    randn so |x| < 6 and no clipping occurs). The device reads 1/4 of the
    fp32 bytes, and the PE runs DoubleRow fp8 matmuls (2 k-rows per
    instruction at 0.5 cycles/row, 2x fp16 throughput; the fp8 DoubleRow
    Ldweights requires the weights k-subtile stride to be a multiple of
    128 - probed against walrus - so rows are stored at their natural 256
    pitch with NO ones column appended). The fp8 quantization costs
    ~1.0e-3 relative error on the loss (measured end to end on the fixed
    inputs; tolerance is 2e-2) because the CORAL loss averages ~65k
    covariance entries, each itself an average of 4096 sample products, so
    per-element quantization noise washes out.
  - Per batch element b on device: partition p of SBUF holds 32 consecutive
    rows of xq[b] (any partition of the n rows is valid for sum_n x x^T,
    and consecutive rows give 4096-byte contiguous DMA runs per partition).
    The PE accumulates, in PSUM, ps0 = S rows 0:128 (all 256 cols) and
    ps1 = S rows 128:256, cols 128:256 only (S is symmetric; the host
    mirrors the lower block). DVE stages PSUM to SBUF as fp16; one DMA per
    batch writes the packed [128, 384] block out.
  - Host (float64): colsum_b = sum_n xq[b] (exact, from the same quantized
    values the device consumed), reassemble S, cov_b = (S_b - colsum_b x
    m_b)/(n-1) with m_b = colsum_b/n, then the tiny masked pairwise CORAL
    reduction (exact mirror of the reference math). ~10 MFLOP on 6.3 MB of
    stats - gather work, like the all-gather + replicated reduction in the
    sharding hint.

Hardware notes:
  - Each batch element owns ONE 2KB PSUM bank ([128, 384] f32; 4 batches +
    warm bank = 5 of 8): no bank reuse, so no claim/fence matmuls and a
    gapless PE stream. PSUM pending-zero is bank-granular, so only the
    first group sets start=True; the second group (cols 256:384) rides the
    same mark with start=False.
  - Most instructions carry at most ONE semaphore wait (PE Matmult/
    Ldweights, DMA descriptors): x tiles get dedicated SBUF slots (no
    reuse -> x DMAs never wait); each epilogue is a SINGLE DVE cast so the
    store has one producer. Early stores ride SWDGE (gpsimd, off the load
    path - HWDGE stores measurably steal load bandwidth mid-stream); the
    LAST store uses the then-idle sync HWDGE ring (~0.3us vs ~3us). A JSON
    post-pass splits Tile's multi-wait kernel-tail Drains into single-wait
    chains and hoists the HWDGE store's ring-lane wait into a Drain.
  - DMA is ~20-24 GB/s per engine and favors large per-partition runs:
    middle batches load as one 8KB-run chunk; batch 0 leads small (early
    PE start), the last batch trails with a small chunk (short PE tail).
  - The PE clock is HAM-gated (~0.94 GHz until ~5us of cumulative matmul
    activity): a warm-up burst on a memset constant opens the gate around
    the time the first chunks land, so most of the stream runs at 2.4 GHz.
"""

import sys

import ml_dtypes
import numpy as np

if "/opt/trn_rl_repo" not in sys.path:
    sys.path.insert(0, "/opt/trn_rl_repo")

import concourse.bass as bass
import concourse.mybir as mybir
import concourse.tile as tile

BZ, N, D = 32, 4096, 256
NCORES = 8
BPC = BZ // NCORES  # batch elements per core
P = 128  # partitions


def build_nc(bpc=BPC, n=N, d=D, kc=16, warmup=10, warmn=512, xp_bufs=None):
    """Per-core Bass module: raw S blocks for `bpc` batch elements.

    Input "x": host-prepared fp8e4 [bpc, n, d].
    Output "outs": fp16 [bpc, 128, 384] packed per-batch blocks
    S[0:128, 0:256] ++ S[128:256, 128:256].
    """
    assert n % P == 0 and d == 2 * P
    kt = n // P  # k-tiles of 128 rows
    assert kt % kc == 0 and kc % 2 == 0

    # The DMA path is packet-rate limited (~85 packets/us; one packet per
    # partition-run), so middle batches load as ONE 8KB-run chunk each.
    # Batch 0 leads with small chunks (fast first descriptor issue + early
    # PE start); the last batch trails with small chunks so the PE tail
    # after the final packet is only ~4 k-pairs.
    def chunk_split(b):
        if b == 0:
            return [kc // 2, kc // 2, kc]
        if b == bpc - 1:
            # One small trailing chunk: the PE tail after the final packet
            # is ~4 k-pairs, and 6KB runs keep the DMA packets efficient.
            return [kc + kc // 2, kc // 2]
        return [2 * kc]

    if xp_bufs is None:
        # One slot per chunk-load: x-tile slots are never reused, so x DMAs
        # never need a slot-release wait (DMAs also carry at most one wait).
        xp_bufs = sum(len(chunk_split(b)) for b in range(bpc))

    nc = bass.Bass(trn_type="TRN2", enable_partition_id=False)
    f32 = mybir.dt.float32
    f16 = mybir.dt.float16
    f8 = mybir.dt.float8e4
    x = nc.dram_tensor("x", [bpc, n, d], f8, kind="ExternalInput")
    w0, w1 = d, d // 2
    # fp16 stats output: S diag ~n gives fp16 abs err ~2 -> cov err ~5e-4 per
    # diag entry, which averages out to ~1e-5 relative on the loss.
    outs = nc.dram_tensor("outs", [bpc, P, w0 + w1], f16, kind="ExternalOutput")
    DR = mybir.MatmulPerfMode.DoubleRow

    with tile.TileContext(nc) as tc:
        with (
            tc.tile_pool(name="xp", bufs=xp_bufs) as xp,
            tc.tile_pool(name="op", bufs=bpc) as op,
            tc.tile_pool(name="constp", bufs=1) as constp,
            tc.tile_pool(name="psp", bufs=bpc, space="PSUM") as psp,
            tc.tile_pool(name="warmp", bufs=1, space="PSUM") as warmp,
        ):
            # Constant operand for warm-up matmuls (DVE memset: cheap, runs
            # during the framework preamble).
            wrm = constp.tile([P, warmn], f16)
            nc.vector.memset(wrm[:, :], 1.0)

            # HAM warm-up: a short burst keeps the PE busy through the DMA
            # descriptor issue + first chunk flight time, ramping the clock
            # gate; the real stream continues the activity so the gate opens
            # (2.4 GHz) shortly into batch 0.
            wps = warmp.tile([1, warmn], f32)
            for _ in range(warmup):
                nc.tensor.matmul(
                    wps[0:1, :], wrm[:, 0:1], wrm[:, 0:warmn],
                    start=True, stop=True, skip_group_check=True,
                )

            # Issue ALL x loads up front: each gets a dedicated SBUF slot and
            # has no dependencies, and the Sync HWDGE ring is FIFO - a store
            # emitted between loads would block later loads behind its wait.
            xts = {}
            nload = 0
            for b in range(bpc):
                k0 = 0
                for c, kcc in enumerate(chunk_split(b)):
                    xt = xp.tile([P, kcc, d], f8, tag=f"xt{kcc}",
                                 name=f"xt_{b}_{c}")
                    # Partition p holds consecutive rows -> contiguous DMA.
                    src = x[b].rearrange("(p k) e -> p k e", p=P)[
                        :, k0 : k0 + kcc, :
                    ]
                    # Alternate chunks across TWO HWDGE rings (sync=SP and
                    # scalar=Activation): one ring's descriptor feed keeps
                    # the 16 DMA engines only ~80% busy.
                    ring = nc.sync if nload % 2 == 0 else nc.scalar
                    ring.dma_start(out=xt[:, :, :], in_=src)
                    nload += 1
                    xts[b, c] = xt
                    k0 += kcc

            def emit_kloop(b):
                # One PSUM bank per batch (4 batches + warm bank = 5 of 8):
                # no bank reuse, so no claim/fence matmuls and no inter-batch
                # PE bubble. ps[:, 0:256] accumulates S[0:128, :]; ps[:,
                # 256:384] accumulates S[128:256, 128:256]. start=True on the
                # FIRST group only: PSUM pending-zero is bank-granular, so it
                # covers the second group's region too, whose first write
                # then zero-substitutes (start=False always on group 2, which
                # also needs skip_group_check since the group tracker wants a
                # start).
                ps = psp.tile([P, w0 + w1], f32, tag="ps", name=f"ps_{b}")
                kk = 0
                kpairs = kt // 2
                for c, kcc in enumerate(chunk_split(b)):
                    xt = xts[b, c]
                    for k in range(0, kcc, 2):
                        # fp8 DoubleRow: one instruction contracts 2 k-tiles
                        # (256 rows) at ~2x fp16 throughput.
                        nc.tensor.matmul(
                            ps[:, 0:w0], xt[:, k : k + 2, 0:P],
                            xt[:, k : k + 2, :],
                            start=(kk == 0), stop=(kk == kpairs - 1),
                            perf_mode=DR,
                        )
                        nc.tensor.matmul(
                            ps[:, w0 : w0 + w1], xt[:, k : k + 2, P:d],
                            xt[:, k : k + 2, P:d],
                            start=False, stop=(kk == kpairs - 1),
                            perf_mode=DR, skip_group_check=True,
                        )
                        kk += 1
                return ps

            def emit_epilogue(b, ps):
                ot = op.tile([P, w0 + w1], f16, tag="ot", name=f"ot_{b}")
                # ONE cast for the whole bank: a single producer keeps the
                # out-DMA at one sync wait (the HW DMA descriptor limit).
                nc.vector.tensor_copy(ot[:, :], ps[:, :])
                if b < bpc - 1:
                    # Early stores ride SWDGE (gpsimd): they overlap the
                    # input stream, and HWDGE stores measurably steal load
                    # bandwidth from the 16 shared DMA engines mid-stream.
                    nc.gpsimd.dma_start(out=outs[b], in_=ot[:, :])
                else:
                    # The LAST store runs after every load has completed, so
                    # the sync HWDGE ring is free: ~0.3us transfer + cheap
                    # single-wait drain, vs ~3us via SWDGE (transfer + gpsimd
                    # drain). Its extra ring-lane wait is hoisted into a
                    # Drain by the JSON post-pass.
                    nc.sync.dma_start(out=outs[b], in_=ot[:, :])

            for b in range(bpc):
                emit_epilogue(b, emit_kloop(b))

    _install_drain_split(nc)
    return nc


def _split_drain_waits(bir, max_waits=1):
    """Keep every instruction at <= `max_waits` sem waits (the HW sync-wait
    table is tiny). Drains with more waits split into a chain of single-wait
    Drains (Tile's kernel-tail drain waits on every active sem lane at once);
    multi-wait DMACopys (HWDGE store: cast-done + ring-lane-reuse) hoist
    their extra waits into synthetic Drains just before them - the engine is
    in-order, so stalling on the Drain then the DMA is equivalent."""
    for fn in bir["functions"]:
        for blk in fn["blocks"]:
            out = []
            changed = False
            for inst in blk["instructions"]:
                waits = (inst.get("sync_info") or {}).get("on_wait") or []
                if inst.get("opcode") == "Drain" and len(waits) > max_waits:
                    changed = True
                    for wi in range(0, len(waits) - max_waits):
                        clone = {
                            **inst,
                            "name": f"{inst['name']}_w{wi}",
                            "sync_info": {
                                "on_wait": [waits[wi]],
                                "on_update": [],
                            },
                        }
                        out.append(clone)
                    inst = {
                        **inst,
                        "sync_info": {
                            **inst["sync_info"],
                            "on_wait": waits[len(waits) - max_waits :],
                        },
                    }
                elif inst.get("opcode") == "DMACopy" and len(waits) > max_waits:
                    changed = True
                    for wi, w in enumerate(waits[max_waits:]):
                        out.append({
                            "debug": inst.get("debug", 0),
                            "engine": inst["engine"],
                            "ins": [],
                            "outs": [],
                            "name": f"{inst['name']}_hw{wi}",
                            "opcode": "Drain",
                            "sync_info": {"on_wait": [w], "on_update": []},
                        })
                    inst = {
                        **inst,
                        "sync_info": {
                            **inst["sync_info"],
                            "on_wait": waits[:max_waits],
                        },
                    }
                out.append(inst)
            if changed:
                blk["instructions"] = out
    return bir


def _install_drain_split(nc):
    import orjson

    raw = nc.to_json_bytes

    def patched():
        return orjson.dumps(_split_drain_waits(orjson.loads(raw())))

    nc.to_json_bytes = patched


_NC_CACHE = {}


def _get_nc():
    key = (BPC, N, D)
    if key not in _NC_CACHE:
        _NC_CACHE[key] = build_nc()
    return _NC_CACHE[key]


def stats_from_raw(outs_blocks, colsum, n=N, d=D):
    """Device outs [bz, 128, 384] (packed, see build_nc) + host colsum
    [bz, d] -> f64 stats."""
    bz = outs_blocks.shape[0]
    h = d // 2
    o = outs_blocks.astype(np.float64)
    s = np.empty((bz, d, d))
    s[:, :h, :] = o[:, :, 0:d]
    s[:, h:, h:] = o[:, :, d : d + h]
    s[:, h:, :h] = np.swapaxes(o[:, :, h:d], 1, 2)  # symmetry mirror
    m = colsum / n
    covs = (s - colsum[:, :, None] * m[:, None, :]) / (n - 1)
    return m, covs


def coral_from_stats(means, covs, domains, d=D):
    """Masked pairwise CORAL reduction from per-batch stats (float64)."""
    bz = means.shape[0]
    m = means.astype(np.float64)
    ms = (m * m).sum(1)
    md = (ms[:, None] + ms[None, :] - 2.0 * (m @ m.T)) / d
    v = covs.astype(np.float64).reshape(bz, -1)
    cs = (v * v).sum(1)
    g = v @ v.T
    cd = (cs[:, None] + cs[None, :] - 2.0 * g) / (d * d)
    upper = np.triu(np.ones((bz, bz), dtype=bool), k=1)
    mask = upper & (np.asarray(domains)[:, None] != np.asarray(domains)[None, :])
    loss = np.where(mask, md + cd, 0.0).sum()
    num = int(mask.sum())
    if num > 1:
        loss = loss / num
    return np.float32(loss)


def kernel(features, domains, _trace=False):
    from concourse import bass_utils

    feats = np.asarray(features)
    assert feats.shape == (BZ, N, D)
    xq = np.asarray(feats, dtype=np.float32).astype(ml_dtypes.float8_e4m3)
    # Exact column sums of the SAME quantized values the device consumes.
    colsum = xq.astype(np.float32).sum(axis=1, dtype=np.float64)  # [bz, d]
    nc = _get_nc()
    xqc = xq.reshape(NCORES, BPC, N, D)
    in_maps = [{"x": xqc[c]} for c in range(NCORES)]
    res = bass_utils.run_bass_kernel_spmd(
        nc, in_maps, core_ids=list(range(NCORES)), trace=_trace
    )
    blocks = np.concatenate([r["outs"] for r in res.results], axis=0)
    means, covs = stats_from_raw(blocks, colsum)
    out = coral_from_stats(means, covs, domains)
    if _trace:
        return out, res
    return out


# revision 20
# speedup vs baseline: 1.0839x; 1.0839x over previous
"""CORAL loss kernel for Trainium2 (8 NeuronCores, Bass/Tile).

Strategy (data-parallel over bz, per sharding hint):
  - Shard features [32, 4096, 256] along bz: 4 batch elements per core.
  - Host casts features to fp8 e4m3 (TRN float8e4, max +-240; values are
    randn so |x| < 6 and no clipping occurs). The device reads 1/4 of the
    fp32 bytes, and the PE runs DoubleRow fp8 matmuls (2 k-rows per
    instruction at 0.5 cycles/row, 2x fp16 throughput; the fp8 DoubleRow
    Ldweights requires the weights k-subtile stride to be a multiple of
    128 - probed against walrus - so rows are stored at their natural 256
    pitch with NO ones column appended). The fp8 quantization costs
    ~1.0e-3 relative error on the loss (measured end to end on the fixed
    inputs; tolerance is 2e-2) because the CORAL loss averages ~65k
    covariance entries, each itself an average of 4096 sample products, so
    per-element quantization noise washes out.
  - Per batch element b on device: partition p of SBUF holds 32 consecutive
    rows of xq[b] (any partition of the n rows is valid for sum_n x x^T,
    and consecutive rows give 4096-byte contiguous DMA runs per partition).
    The PE accumulates, in PSUM, ps0 = S rows 0:128 (all 256 cols) and
    ps1 = S rows 128:256, cols 128:256 only (S is symmetric; the host
    mirrors the lower block). DVE stages PSUM to SBUF as fp16; one DMA per
    batch writes the packed [128, 384] block out.
  - Host (float64): colsum_b = sum_n xq[b] (exact, from the same quantized
    values the device consumed), reassemble S, cov_b = (S_b - colsum_b x
    m_b)/(n-1) with m_b = colsum_b/n, then the tiny masked pairwise CORAL
    reduction (exact mirror of the reference math). ~10 MFLOP on 6.3 MB of
    stats - gather work, like the all-gather + replicated reduction in the
    sharding hint.

Hardware notes:
  - Each batch element owns ONE 2KB PSUM bank ([128, 384] f32; 4 batches +
    warm bank = 5 of 8): no bank reuse, so no claim/fence matmuls and a
    gapless PE stream. PSUM pending-zero is bank-granular, so only the
    first group sets start=True; the second group (cols 256:384) rides the
    same mark with start=False.
  - Most instructions carry at most ONE semaphore wait (PE Matmult/
    Ldweights, DMA descriptors): x tiles get dedicated SBUF slots (no
    reuse -> x DMAs never wait); each epilogue is a SINGLE DVE cast so the
    store has one producer. Early stores ride SWDGE (gpsimd, off the load
    path - HWDGE stores measurably steal load bandwidth mid-stream); the
    LAST store uses the then-idle sync HWDGE ring (~0.3us vs ~3us). A JSON
    post-pass splits Tile's multi-wait kernel-tail Drains into single-wait
    chains and hoists the HWDGE store's ring-lane wait into a Drain.
  - DMA is ~20-24 GB/s per engine and favors large per-partition runs:
    middle batches load as one 8KB-run chunk; batch 0 leads small (early
    PE start), the last batch trails with a small chunk (short PE tail).
  - The PE clock is HAM-gated (~0.94 GHz until ~5us of cumulative matmul
    activity): a warm-up burst on a memset constant opens the gate around
    the time the first chunks land, so most of the stream runs at 2.4 GHz.
"""

import sys

import ml_dtypes
import numpy as np

if "/opt/trn_rl_repo" not in sys.path:
    sys.path.insert(0, "/opt/trn_rl_repo")

import concourse.bass as bass
import concourse.mybir as mybir
import concourse.tile as tile

BZ, N, D = 32, 4096, 256
NCORES = 8
BPC = BZ // NCORES  # batch elements per core
P = 128  # partitions


def build_nc(bpc=BPC, n=N, d=D, kc=16, warmup=16, warmn=512, xp_bufs=None):
    """Per-core Bass module: raw S blocks for `bpc` batch elements.

    Input "x": host-prepared fp8e4 [bpc, n, d].
    Output "outs": fp16 [bpc, 128, 384] packed per-batch blocks
    S[0:128, 0:256] ++ S[128:256, 128:256].
    """
    assert n % P == 0 and d == 2 * P
    kt = n // P  # k-tiles of 128 rows
    assert kt % kc == 0 and kc % 2 == 0

    # The DMA path is packet-rate limited (~85 packets/us; one packet per
    # partition-run), so middle batches load as ONE 8KB-run chunk each.
    # Batch 0 leads with small chunks (fast first descriptor issue + early
    # PE start); the last batch trails with small chunks so the PE tail
    # after the final packet is only ~4 k-pairs.
    def chunk_split(b):
        if b == 0:
            return [kc // 2, kc // 2, kc]
        if b == bpc - 1:
            # One small trailing chunk: the PE tail after the final packet
            # is ~4 k-pairs, and 6KB runs keep the DMA packets efficient.
            return [kc + kc // 2, kc // 2]
        return [2 * kc]

    if xp_bufs is None:
        # One slot per chunk-load: x-tile slots are never reused, so x DMAs
        # never need a slot-release wait (DMAs also carry at most one wait).
        xp_bufs = sum(len(chunk_split(b)) for b in range(bpc))

    nc = bass.Bass(trn_type="TRN2", enable_partition_id=False)
    f32 = mybir.dt.float32
    f16 = mybir.dt.float16
    f8 = mybir.dt.float8e4
    x = nc.dram_tensor("x", [bpc, n, d], f8, kind="ExternalInput")
    w0, w1 = d, d // 2
    # fp16 stats output: S diag ~n gives fp16 abs err ~2 -> cov err ~5e-4 per
    # diag entry, which averages out to ~1e-5 relative on the loss.
    outs = nc.dram_tensor("outs", [bpc, P, w0 + w1], f16, kind="ExternalOutput")
    DR = mybir.MatmulPerfMode.DoubleRow

    with tile.TileContext(nc) as tc:
        with (
            tc.tile_pool(name="xp", bufs=xp_bufs) as xp,
            tc.tile_pool(name="op", bufs=bpc) as op,
            tc.tile_pool(name="constp", bufs=1) as constp,
            tc.tile_pool(name="psp", bufs=bpc, space="PSUM") as psp,
            tc.tile_pool(name="warmp", bufs=1, space="PSUM") as warmp,
        ):
            # Constant operand for warm-up matmuls (DVE memset: cheap, runs
            # during the framework preamble).
            wrm = constp.tile([P, warmn], f16)
            nc.vector.memset(wrm[:, :], 1.0)

            # HAM warm-up: a short burst keeps the PE busy through the DMA
            # descriptor issue + first chunk flight time, ramping the clock
            # gate; the real stream continues the activity so the gate opens
            # (2.4 GHz) shortly into batch 0.
            wps = warmp.tile([1, warmn], f32)
            for _ in range(warmup):
                nc.tensor.matmul(
                    wps[0:1, :], wrm[:, 0:1], wrm[:, 0:warmn],
                    start=True, stop=True, skip_group_check=True,
                )

            # Issue ALL x loads up front: each gets a dedicated SBUF slot and
            # has no dependencies, and the Sync HWDGE ring is FIFO - a store
            # emitted between loads would block later loads behind its wait.
            xts = {}
            for b in range(bpc):
                k0 = 0
                for c, kcc in enumerate(chunk_split(b)):
                    xt = xp.tile([P, kcc, d], f8, tag=f"xt{kcc}",
                                 name=f"xt_{b}_{c}")
                    # Partition p holds consecutive rows -> contiguous DMA.
                    src = x[b].rearrange("(p k) e -> p k e", p=P)[
                        :, k0 : k0 + kcc, :
                    ]
                    nc.sync.dma_start(out=xt[:, :, :], in_=src)
                    xts[b, c] = xt
                    k0 += kcc

            def emit_kloop(b):
                # One PSUM bank per batch (4 batches + warm bank = 5 of 8):
                # no bank reuse, so no claim/fence matmuls and no inter-batch
                # PE bubble. ps[:, 0:256] accumulates S[0:128, :]; ps[:,
                # 256:384] accumulates S[128:256, 128:256]. start=True on the
                # FIRST group only: PSUM pending-zero is bank-granular, so it
                # covers the second group's region too, whose first write
                # then zero-substitutes (start=False always on group 2, which
                # also needs skip_group_check since the group tracker wants a
                # start).
                ps = psp.tile([P, w0 + w1], f32, tag="ps", name=f"ps_{b}")
                kk = 0
                kpairs = kt // 2
                for c, kcc in enumerate(chunk_split(b)):
                    xt = xts[b, c]
                    for k in range(0, kcc, 2):
                        # fp8 DoubleRow: one instruction contracts 2 k-tiles
                        # (256 rows) at ~2x fp16 throughput.
                        nc.tensor.matmul(
                            ps[:, 0:w0], xt[:, k : k + 2, 0:P],
                            xt[:, k : k + 2, :],
                            start=(kk == 0), stop=(kk == kpairs - 1),
                            perf_mode=DR,
                        )
                        nc.tensor.matmul(
                            ps[:, w0 : w0 + w1], xt[:, k : k + 2, P:d],
                            xt[:, k : k + 2, P:d],
                            start=False, stop=(kk == kpairs - 1),
                            perf_mode=DR, skip_group_check=True,
                        )
                        kk += 1
                return ps

            def emit_epilogue(b, ps):
                ot = op.tile([P, w0 + w1], f16, tag="ot", name=f"ot_{b}")
                # ONE cast for the whole bank: a single producer keeps the
                # out-DMA at one sync wait (the HW DMA descriptor limit).
                nc.vector.tensor_copy(ot[:, :], ps[:, :])
                if b < bpc - 1:
                    # Early stores ride SWDGE (gpsimd): they overlap the
                    # input stream, and HWDGE stores measurably steal load
                    # bandwidth from the 16 shared DMA engines mid-stream.
                    nc.gpsimd.dma_start(out=outs[b], in_=ot[:, :])
                else:
                    # The LAST store runs after every load has completed, so
                    # the sync HWDGE ring is free: ~0.3us transfer + cheap
                    # single-wait drain, vs ~3us via SWDGE (transfer + gpsimd
                    # drain). Its extra ring-lane wait is hoisted into a
                    # Drain by the JSON post-pass.
                    nc.sync.dma_start(out=outs[b], in_=ot[:, :])

            for b in range(bpc):
                emit_epilogue(b, emit_kloop(b))

    _install_drain_split(nc)
    return nc


def _split_drain_waits(bir, max_waits=1):
    """Keep every instruction at <= `max_waits` sem waits (the HW sync-wait
    table is tiny). Drains with more waits split into a chain of single-wait
    Drains (Tile's kernel-tail drain waits on every active sem lane at once);
    multi-wait DMACopys (HWDGE store: cast-done + ring-lane-reuse) hoist
    their extra waits into synthetic Drains just before them - the engine is
    in-order, so stalling on the Drain then the DMA is equivalent."""
    for fn in bir["functions"]:
        for blk in fn["blocks"]:
            out = []
            changed = False
            for inst in blk["instructions"]:
                waits = (inst.get("sync_info") or {}).get("on_wait") or []
                if inst.get("opcode") == "Drain" and len(waits) > max_waits:
                    changed = True
                    for wi in range(0, len(waits) - max_waits):
                        clone = {
                            **inst,
                            "name": f"{inst['name']}_w{wi}",
                            "sync_info": {
                                "on_wait": [waits[wi]],
                                "on_update": [],
                            },
                        }
                        out.append(clone)
                    inst = {
                        **inst,
                        "sync_info": {
                            **inst["sync_info"],
                            "on_wait": waits[len(waits) - max_waits :],
                        },
                    }
                elif inst.get("opcode") == "DMACopy" and len(waits) > max_waits:
                    changed = True
                    for wi, w in enumerate(waits[max_waits:]):
                        out.append({
                            "debug": inst.get("debug", 0),
                            "engine": inst["engine"],
                            "ins": [],
                            "outs": [],
                            "name": f"{inst['name']}_hw{wi}",
                            "opcode": "Drain",
                            "sync_info": {"on_wait": [w], "on_update": []},
                        })
                    inst = {
                        **inst,
                        "sync_info": {
                            **inst["sync_info"],
                            "on_wait": waits[:max_waits],
                        },
                    }
                out.append(inst)
            if changed:
                blk["instructions"] = out
    return bir


def _install_drain_split(nc):
    import orjson

    raw = nc.to_json_bytes

    def patched():
        return orjson.dumps(_split_drain_waits(orjson.loads(raw())))

    nc.to_json_bytes = patched


_NC_CACHE = {}


def _get_nc():
    key = (BPC, N, D)
    if key not in _NC_CACHE:
        _NC_CACHE[key] = build_nc()
    return _NC_CACHE[key]


def stats_from_raw(outs_blocks, colsum, n=N, d=D):
    """Device outs [bz, 128, 384] (packed, see build_nc) + host colsum
    [bz, d] -> f64 stats."""
    bz = outs_blocks.shape[0]
    h = d // 2
    o = outs_blocks.astype(np.float64)
    s = np.empty((bz, d, d))
    s[:, :h, :] = o[:, :, 0:d]
    s[:, h:, h:] = o[:, :, d : d + h]
    s[:, h:, :h] = np.swapaxes(o[:, :, h:d], 1, 2)  # symmetry mirror
    m = colsum / n
    covs = (s - colsum[:, :, None] * m[:, None, :]) / (n - 1)
    return m, covs


def coral_from_stats(means, covs, domains, d=D):
    """Masked pairwise CORAL reduction from per-batch stats (float64)."""
    bz = means.shape[0]
    m = means.astype(np.float64)
    ms = (m * m).sum(1)
    md = (ms[:, None] + ms[None, :] - 2.0 * (m @ m.T)) / d
    v = covs.astype(np.float64).reshape(bz, -1)
    cs = (v * v).sum(1)
    g = v @ v.T
    cd = (cs[:, None] + cs[None, :] - 2.0 * g) / (d * d)
    upper = np.triu(np.ones((bz, bz), dtype=bool), k=1)
    mask = upper & (np.asarray(domains)[:, None] != np.asarray(domains)[None, :])
    loss = np.where(mask, md + cd, 0.0).sum()
    num = int(mask.sum())
    if num > 1:
        loss = loss / num
    return np.float32(loss)


def kernel(features, domains, _trace=False):
    from concourse import bass_utils

    feats = np.asarray(features)
    assert feats.shape == (BZ, N, D)
    xq = np.asarray(feats, dtype=np.float32).astype(ml_dtypes.float8_e4m3)
    # Exact column sums of the SAME quantized values the device consumes.
    colsum = xq.astype(np.float32).sum(axis=1, dtype=np.float64)  # [bz, d]
    nc = _get_nc()
    xqc = xq.reshape(NCORES, BPC, N, D)
    in_maps = [{"x": xqc[c]} for c in range(NCORES)]
    res = bass_utils.run_bass_kernel_spmd(
        nc, in_maps, core_ids=list(range(NCORES)), trace=_trace
    )
    blocks = np.concatenate([r["outs"] for r in res.results], axis=0)
    means, covs = stats_from_raw(blocks, colsum)
    out = coral_from_stats(means, covs, domains)
    if _trace:
        return out, res
    return out
